# revision 11
# baseline (speedup 1.0000x reference)
"""GAT (2-layer graph attention + output projection) on 8 Trainium2 NeuronCores.

v2. Sharding: destination nodes partitioned across cores (contiguous ranges of
6250), then *permuted within each core* by a balanced bin-packing so that every
128-node chunk has <=768 incoming lo-edges and <=384 hi-edges (lo/hi = source
row < / >= 32768 in the padded global table; int16 gather indices force the
split). This gives a uniform T_LO=6 / T_HI=3 tile structure (441 tiles/core,
~12% pad) vs the naive layout's (6,4) = 490 tiles.

Per layer: ft = x @ [W | wr] per local chunk (bf16, fed by transposed x tables
so no PE transposes); er (4 attention cols) stays in SBUF. ft tables are
AllGathered, each core dma_gathers ft[src] rows for its edges (two gathers,
lo/hi, striped over 2 SWDGE queues). Per-edge er[dst] is reconstructed on-chip:
dstrel values are partition-broadcast + compared against a partition-iota to
build transposed one-hots, and a [128x128]@[128x4] matmul per tile broadcasts
the chunk's er into edge slots -- this removes the v1 per-edge er dma_gather
(half of all descriptors). Segment softmax runs without max subtraction;
messages scatter into per-chunk PSUM via one-hot matmuls; the output head is
fused into the layer-2 epilogue using the transposed x1/x2 tiles directly.
"""
import math
import os
import numpy as np

import concourse.bass as bass
import concourse.tile as tile
from concourse import bacc, mybir
from concourse.bass_utils import run_bass_kernel_spmd
from concourse.masks import make_identity
from contextlib import ExitStack

F32 = mybir.dt.float32
BF16 = mybir.dt.bfloat16
I16 = mybir.dt.int16

NCORE = 8
H, Dh = 4, 32
NEG = 0.1
HALF = 32768
P = 128
PAD_DST = 1000.0


# ----------------------------------------------------------------------------
# host-side graph preprocessing
# ----------------------------------------------------------------------------
def _pack_bins(deg_lo, deg_hi, nbins, cap_lo, cap_hi):
    """Greedy balanced packing of nodes into nbins chunks of <=128 nodes with
    per-chunk lo/hi edge caps. Returns bin id per node, or None if infeasible."""
    n = len(deg_lo)
    order = np.argsort(-(deg_lo + deg_hi), kind="stable")
    rem_lo = np.full(nbins, cap_lo, np.int64)
    rem_hi = np.full(nbins, cap_hi, np.int64)
    cnt = np.zeros(nbins, np.int64)
    binid = np.full(n, -1, np.int64)
    for i in order:
        l, h = deg_lo[i], deg_hi[i]
        feas = (rem_lo >= l) & (rem_hi >= h) & (cnt < P)
        if not feas.any():
            return None
        score = np.where(feas, rem_lo + rem_hi, -1)
        b = int(np.argmax(score))
        binid[i] = b
        rem_lo[b] -= l
        rem_hi[b] -= h
        cnt[b] += 1
    return binid


def _edge_plan(src, dst, w, n_nodes, nloc, nlp, G):
    """Balanced-packed edge plan. Returns per-core gather/scatter arrays and
    the node permutation (slot_of: global node -> padded global row)."""
    nch = nlp // P
    npad = NCORE * nlp
    core_of_node = np.minimum(np.arange(n_nodes) // nloc, NCORE - 1)
    loc_of_node = np.arange(n_nodes) - core_of_node * nloc

    # core 5 straddles the HALF boundary: pin its first `pin` nodes (by local
    # id) to chunks [0, pin//P) so lo/hi classification is permutation-free.
    pin_core = HALF // nlp            # 5
    pin = HALF - pin_core * nlp       # 1408
    lo_src_bound = pin_core * nloc + pin   # global id threshold for lo edges

    edge_lo = src < lo_src_bound
    dst_core = dst // nloc

    # per-core packing -> slot_of (global node id -> padded global row)
    slot_of = np.zeros(n_nodes, np.int64)
    T_LO_req, T_HI_req = 6, 3
    for c in range(NCORE):
        lo_ = c * nloc
        hi_ = min((c + 1) * nloc, n_nodes)
        nn = hi_ - lo_
        deg_lo = np.bincount(dst[(dst_core == c) & edge_lo] - lo_, minlength=nn)
        deg_hi = np.bincount(dst[(dst_core == c) & ~edge_lo] - lo_, minlength=nn)
        for caps in ((6, 3), (7, 4), (8, 5)):
            cl, chh = caps[0] * P, caps[1] * P
            if c == pin_core:
                nb0 = pin // P
                b0 = _pack_bins(deg_lo[:pin], deg_hi[:pin], nb0, cl, chh)
                b1 = _pack_bins(deg_lo[pin:], deg_hi[pin:], nch - nb0, cl, chh)
                if b0 is None or b1 is None:
                    continue
                binid = np.concatenate([b0, b1 + nb0])
            else:
                binid = _pack_bins(deg_lo, deg_hi, nch, cl, chh)
                if binid is None:
                    continue
            T_LO_req = max(T_LO_req, caps[0])
            T_HI_req = max(T_HI_req, caps[1])
            break
        else:
            raise RuntimeError("packing failed")
        # slots: nodes of bin b occupy rows b*P.. in assignment order
        slot = np.zeros(nn, np.int64)
        for b in range(nch):
            members = np.where(binid == b)[0]
            slot[members] = b * P + np.arange(len(members))
        slot_of[lo_:hi_] = c * nlp + slot

    T_LO, T_HI = T_LO_req, T_HI_req
    T = T_LO + T_HI
    srcpad = slot_of[src]
    dstpad = slot_of[dst]

    groups = [G] * (nch // G) + ([nch % G] if nch % G else [])
    S_tot = sum(g * T for g in groups)
    LO_tot = sum(g * T_LO for g in groups) * P
    HI_tot = sum(g * T_HI for g in groups) * P

    def wrap(flat):
        a = flat.reshape(-1, 16).T.copy()
        return np.tile(a, (8, 1)).astype(np.int16)

    out = []
    for c in range(NCORE):
        m = dst_core == c
        e_ids = np.where(m)[0]
        e_chunk = (dstpad[e_ids] - c * nlp) // P
        e_rel = (dstpad[e_ids] - c * nlp) % P
        e_lo = edge_lo[e_ids]

        srclo = np.zeros(LO_tot, np.int64)
        srchi = np.zeros(HI_tot, np.int64)
        dstrel = np.full((P, S_tot), PAD_DST, np.float32)
        wcol = np.zeros((P, S_tot), np.float32)

        # bucket edges per (chunk, lo/hi)
        order = np.argsort(e_chunk * 2 + (~e_lo).astype(np.int64), kind="stable")
        eo = e_ids[order]
        co = e_chunk[order]
        ro = e_rel[order]
        loo = e_lo[order]

        s_off = lo_off = hi_off = 0
        ch0 = 0
        ck_lo_idx = [np.where((co == ch) & loo)[0] for ch in range(nch)]
        ck_hi_idx = [np.where((co == ch) & ~loo)[0] for ch in range(nch)]

        for g in groups:
            for cg in range(g):
                ch = ch0 + cg
                for is_lo, idxs, Treg, roff, arr, rbase in (
                    (True, ck_lo_idx[ch], T_LO, lo_off, srclo, 0),
                    (False, ck_hi_idx[ch], T_HI, hi_off, srchi, HALF),
                ):
                    for j, ei in enumerate(idxs):
                        t, p = j // P, j % P
                        if is_lo:
                            i_flat = roff + (cg * Treg + t) * P + p
                            arr[i_flat] = srcpad[eo[ei]] - rbase
                            s = s_off + cg * T_LO + t
                        else:
                            i_flat = roff + (cg * Treg + t) * P + p
                            arr[i_flat] = srcpad[eo[ei]] - rbase
                            s = s_off + g * T_LO + cg * T_HI + t
                        dstrel[p, s] = float(ro[ei])
                        wcol[p, s] = w[eo[ei]]
            s_off += g * T
            lo_off += g * T_LO * P
            hi_off += g * T_HI * P
            ch0 += g

        import ml_dtypes
        out.append(dict(
            srclo_w=wrap(srclo), srchi_w=wrap(srchi),
            dstrel=dstrel.astype(ml_dtypes.bfloat16), wcol=wcol,
            dstrelT=np.ascontiguousarray(dstrel.T).reshape(1, -1).astype(
                ml_dtypes.bfloat16),
        ))
    return dict(T_LO=T_LO, T_HI=T_HI, groups=groups, cores=out, slot_of=slot_of,
                S_tot=S_tot, LO_tot=LO_tot, HI_tot=HI_tot)


# ----------------------------------------------------------------------------
# device program
# ----------------------------------------------------------------------------
def _build_program(n_nodes, nloc, nlp, kin1, plan):
    T_LO, T_HI, groups = plan["T_LO"], plan["T_HI"], plan["groups"]
    T = T_LO + T_HI
    nch = nlp // P
    npad = NCORE * nlp

    nswq = int(os.environ.get("KNSWQ", "2"))
    nc = bacc.Bacc("TRN2", target_bir_lowering=False, debug=False,
                   num_devices=NCORE, num_swdge_queues=nswq)

    # ---- I/O ----
    # layer-1 ft is computed REPLICATED (every core computes the full table
    # from the full feature matrix) -- 43us of PE work replaces a ~730us
    # AllGather. x0T is the full slot-ordered feature table (same on every
    # core); x0Tl is this core's slice (only used for the tiny er1 pass,
    # which must read core-local columns in an SPMD-uniform way).
    x0T = nc.dram_tensor("x0T", [kin1, NCORE * nlp], BF16,
                         kind="ExternalInput")
    x0Tl = nc.dram_tensor("x0Tl", [kin1, nlp], BF16, kind="ExternalInput")
    w1e = nc.dram_tensor("w1e", [kin1, 132], BF16, kind="ExternalInput")
    w2e = nc.dram_tensor("w2e", [128, 132], BF16, kind="ExternalInput")
    wout = nc.dram_tensor("wout", [256, 128], BF16, kind="ExternalInput")
    boutr = nc.dram_tensor("boutr", [1, 128], BF16, kind="ExternalInput")
    al1r = nc.dram_tensor("al1r", [P, T * P], BF16, kind="ExternalInput")
    al2r = nc.dram_tensor("al2r", [P, T * P], BF16, kind="ExternalInput")
    iota = nc.dram_tensor("iota", [P, P], BF16, kind="ExternalInput")
    iotac = nc.dram_tensor("iotac", [P, 1], BF16, kind="ExternalInput")
    dstrelT = nc.dram_tensor("dstrelT", [1, plan["S_tot"] * P], BF16,
                             kind="ExternalInput")
    onesr = nc.dram_tensor("onesr", [1, 128], BF16, kind="ExternalInput")
    srclo = nc.dram_tensor("srclo", [P, plan["LO_tot"] // 16], I16, kind="ExternalInput")
    srchi = nc.dram_tensor("srchi", [P, plan["HI_tot"] // 16], I16, kind="ExternalInput")
    dstrel = nc.dram_tensor("dstrel", [P, plan["S_tot"]], BF16, kind="ExternalInput")
    wcol = nc.dram_tensor("wcol", [P, plan["S_tot"]], F32, kind="ExternalInput")
    out_local = nc.dram_tensor("out_local", [nlp, 128], F32, kind="ExternalOutput")

    # ---- internal DRAM ----
    # x1T is written in NSPL column blocks; each block is AllGathered as soon
    # as its chunks complete during the layer-1 edge phase, overlapping the
    # collective with edge compute. ft2 is then rebuilt REPLICATED on every
    # core from the gathered x1T blocks (43us of PE work), so ftf2 is local.
    ftf1 = nc.dram_tensor("ft1_full", [npad, 128], BF16)
    ftf2 = nc.dram_tensor("ft2_full", [npad, 128], BF16)
    nspl = int(os.environ.get("KNSPL", "4"))
    # distribute the edge-phase groups into nspl consecutive blocks with
    # roughly equal chunk counts
    grp_block = []
    blk_groups = [[] for _ in range(nspl)]
    acc = 0
    tot_ch = sum(groups)
    for gi, g in enumerate(groups):
        k = min(nspl - 1, acc * nspl // tot_ch)
        grp_block.append(k)
        blk_groups[k].append(gi)
        acc += g
    blk_ch0 = []
    blk_nch = []
    acc = 0
    for k in range(nspl):
        nchk = sum(groups[gi] for gi in blk_groups[k])
        blk_ch0.append(acc)
        blk_nch.append(nchk)
        acc += nchk
    x1T_k = [nc.dram_tensor(f"x1T_{k}", [128, blk_nch[k] * P], BF16)
             for k in range(nspl)]
    x1Tf_k = [nc.dram_tensor(f"x1Tf_{k}", [NCORE * 128, blk_nch[k] * P], BF16,
                             addr_space="Shared") for k in range(nspl)]
    blk_of_ch = []
    for k in range(nspl):
        blk_of_ch += [k] * blk_nch[k]

    rg = [list(range(NCORE))]

    with tile.TileContext(nc) as tc, ExitStack() as ctx:
        consts = ctx.enter_context(tc.tile_pool(name="consts", bufs=1))
        sb = ctx.enter_context(tc.tile_pool(
            name="sb", bufs=int(os.environ.get("KSBUFS", "3"))))
        sb_ft = ctx.enter_context(tc.tile_pool(name="sb_ft", bufs=3))
        ps_ft = ctx.enter_context(tc.tile_pool(name="ps_ft", bufs=2, space="PSUM"))
        ps_e = ctx.enter_context(tc.tile_pool(name="ps_e", bufs=2, space="PSUM"))

        # ---- constants in SBUF ----
        iota_t = consts.tile([P, P], BF16)
        nc.sync.dma_start(iota_t[:], iota[:, :])
        iotac_t = consts.tile([P, 1], BF16)
        nc.sync.dma_start(iotac_t[:], iotac[:, :])
        ones_t = consts.tile([1, 128], BF16)
        nc.sync.dma_start(ones_t[:], onesr[:, :])
        bout_t = consts.tile([1, 128], BF16)
        nc.sync.dma_start(bout_t[:], boutr[:, :])
        w1e_t = [consts.tile([P, 132], BF16, name=f"w1e{k}", tag=f"w1e{k}")
                 for k in range(kin1 // P)]
        for k in range(kin1 // P):
            nc.sync.dma_start(w1e_t[k][:], w1e[k * P:(k + 1) * P, :])
        w2e_t = consts.tile([P, 132], BF16)
        nc.sync.dma_start(w2e_t[:], w2e[:, :])
        wout_t = [consts.tile([P, 128], BF16, name=f"wout{k}", tag=f"wout{k}")
                  for k in range(2)]
        for k in range(2):
            nc.sync.dma_start(wout_t[k][:], wout[k * P:(k + 1) * P, :])
        al_t = []
        for l, src_al in ((0, al1r), (1, al2r)):
            a = consts.tile([P, T * P], BF16, name=f"al{l}", tag=f"al{l}")
            nc.sync.dma_start(a[:], src_al[:, :])
            al_t.append(a)
        er_st = [consts.tile([P, nch, 4], BF16, name=f"er{l}", tag=f"er{l}")
                 for l in (1, 2)]
        ident_t = consts.tile([P, P], F32)
        make_identity(nc, ident_t[:])
        identb_t = consts.tile([P, P], BF16)
        make_identity(nc, identb_t[:])

        def er1_phase(kin, er_tile):
            """er1 for this core's local chunks: x0Tl @ w1e[:, 128:132]."""
            EB = 4
            for cb in range(0, nch, EB):
                nbc = min(EB, nch - cb)
                xts = []
                for k in range(kin // P):
                    xt = sb_ft.tile([P, EB * P], BF16, tag="erxt", bufs=4)
                    nc.sync.dma_start(
                        xt[:, 0:nbc * P],
                        x0Tl[k * P:(k + 1) * P, cb * P:(cb + nbc) * P])
                    xts.append(xt)
                for ci in range(nbc):
                    pse = ps_ft.tile([P, 4], F32, space="PSUM", tag="psf")
                    for k in range(kin // P):
                        nc.tensor.matmul(
                            out=pse[:], lhsT=xts[k][:, ci * P:(ci + 1) * P],
                            rhs=w1e_t[k][:, 128:132],
                            start=(k == 0), stop=(k == kin // P - 1))
                    nc.vector.tensor_copy(er_tile[:, cb + ci, :], pse[:])

        def ft1_full_phase(kin, we_tiles):
            """Full replicated ft1 table: every core computes all npad rows."""
            nchg = npad // P            # 392 global chunks
            BLKC = 14                   # chunks per slab load (28 blocks)
            BCH = 3                     # chunks per PSUM batch (<=512 f32/bank)
            for b0 in range(0, nchg, BLKC):
                nbk = min(BLKC, nchg - b0)
                slabs = []
                for k in range(kin // P):
                    s = sb_ft.tile([P, BLKC * P], BF16, tag=f"slab{k}", bufs=2)
                    nc.sync.dma_start(
                        s[:, 0:nbk * P],
                        x0T[k * P:(k + 1) * P, b0 * P:(b0 + nbk) * P])
                    slabs.append(s)
                for c0 in range(0, nbk, BCH):
                    nb = min(BCH, nbk - c0)
                    psb = ps_ft.tile([P, BCH, 132], F32, space="PSUM",
                                     tag="psb")
                    for ci in range(nb):
                        for k in range(kin // P):
                            nc.tensor.matmul(
                                out=psb[:, ci, :],
                                lhsT=slabs[k][:, (c0 + ci) * P:(c0 + ci + 1) * P],
                                rhs=we_tiles[k][:],
                                start=(k == 0), stop=(k == kin // P - 1))
                    ftb = sb_ft.tile([P, BCH, 128], BF16, tag="ftb")
                    nc.scalar.activation(ftb[:, 0:nb, :], psb[:, 0:nb, 0:128],
                                         mybir.ActivationFunctionType.Copy)
                    r0 = (b0 + c0) * P
                    nc.sync.dma_start(
                        ftf1[r0:r0 + nb * P, :].rearrange(
                            "(c p) f -> p c f", c=nb),
                        ftb[:, 0:nb, :])

        def edge_phase(layer, ftf_d, er_tile, al_rep, do_head=False,
                       fuse_ft2=None):
            estage = int(os.environ.get("KEDGE", "9"))
            s_off = lo_off = hi_off = 0
            ch0 = 0
            for gi, g in enumerate(groups):
                S = g * T
                nlo, nhi = g * T_LO * P, g * T_HI * P
                # --- edge metadata loads ---
                ilo = sb.tile([P, nlo // 16], I16, tag="ilo")
                nc.sync.dma_start(ilo[:], srclo[:, lo_off // 16:(lo_off + nlo) // 16])
                ihi = sb.tile([P, nhi // 16], I16, tag="ihi")
                nc.sync.dma_start(ihi[:], srchi[:, hi_off // 16:(hi_off + nhi) // 16])
                dr = sb.tile([P, S], BF16, tag="dr")
                nc.sync.dma_start(dr[:], dstrel[:, s_off:s_off + S])
                wc = sb.tile([P, S], F32, tag="wc")
                nc.sync.dma_start(wc[:], wcol[:, s_off:s_off + S])
                drT = sb.tile([1, S * P], BF16, tag="drT")
                nc.sync.dma_start(drT[:], dstrelT[:, s_off * P:(s_off + S) * P])

                # --- ft[src] gathers (2 queues) ---
                g_t = sb.tile([P, S, 128], BF16, tag="g_t",
                              bufs=int(os.environ.get("KGBUFS", "3")))
                nq = 1 if os.environ.get("KNOQ2") else nswq
                nc.gpsimd.dma_gather(
                    out_ap=g_t[:, 0:g * T_LO, :], in_ap=ftf_d[:, :],
                    idxs_ap=ilo[:], num_idxs=nlo, num_idxs_reg=nlo, elem_size=128,
                    single_packet=False, queue_num=(2 * gi) % nq,
                )
                nc.gpsimd.dma_gather(
                    out_ap=g_t[:, g * T_LO:S, :], in_ap=ftf_d[HALF:npad, :],
                    idxs_ap=ihi[:], num_idxs=nhi, num_idxs_reg=nhi, elem_size=128,
                    single_packet=False, queue_num=(2 * gi + 1) % nq,
                )

                # --- scatter one-hots (DVE) + transposed one-hots (PE) ---
                build_oh = not os.environ.get("KNOOH")
                if build_oh:
                    oh = sb.tile([P, S, P], BF16, tag="oh")
                    nc.vector.tensor_tensor(
                        out=oh[:],
                        in0=iota_t[:, None, :].to_broadcast([P, S, P]),
                        in1=dr[:, :, None].to_broadcast([P, S, P]),
                        op=mybir.AluOpType.is_equal,
                    )
                    drTb = sb.tile([P, S, P], BF16, tag="drTb")
                    nc.gpsimd.partition_broadcast(
                        drTb[:].rearrange("p s q -> p (s q)"), drT[:])
                    ohT = sb.tile([P, S, P], BF16, tag="ohT")
                    nc.vector.tensor_tensor(
                        out=ohT[:].rearrange("p s q -> p (s q)"),
                        in0=drTb[:].rearrange("p s q -> p (s q)"),
                        in1=iotac_t[:, :].to_broadcast([P, S * P]),
                        op=mybir.AluOpType.is_equal,
                    )

                if estage < 2:
                    xc0 = sb.tile([P, 128], F32, tag="xck")
                    nc.vector.tensor_copy(xc0[:], g_t[:, 0, :])
                    nc.sync.dma_start(out_local[ch0 * P:(ch0 + 1) * P, :], xc0[:])
                    s_off += S; lo_off += nlo; hi_off += nhi; ch0 += g
                    continue

                # --- er broadcast into edge slots: ere[p, s*4+h] ---
                ere_ps = ps_e.tile([P, S * 4], F32, space="PSUM", tag="ere")
                for cg in range(g):
                    for t in range(T):
                        if t < T_LO:
                            s = cg * T_LO + t
                        else:
                            s = g * T_LO + cg * T_HI + (t - T_LO)
                        nc.tensor.matmul(
                            out=ere_ps[:, s * 4:(s + 1) * 4],
                            lhsT=ohT[:, s, :], rhs=er_tile[:, ch0 + cg, :],
                            start=True, stop=True,
                        )

                # --- el = <ft, al> per head (elp scratch aliases msg cols) ---
                msg = sb.tile([P, S, 132], BF16, tag="msg",
                              bufs=int(os.environ.get("KGBUFS", "3")))
                elp = msg[:, :, 0:128]
                for cg in range(g):
                    nc.vector.tensor_mul(
                        elp[:, cg * T:(cg + 1) * T, :],
                        g_t[:, cg * T:(cg + 1) * T, :],
                        al_rep[:].rearrange("p (t j) -> p t j", j=P),
                    )
                el = sb.tile([P, S, 4], F32, tag="el")
                nc.vector.reduce_sum(
                    el[:],
                    elp[:].rearrange("p s (h d) -> p s h d", h=H),
                    axis=mybir.AxisListType.X,
                )
                # logit = lrelu(el + er) * w ; au = exp(logit)
                nc.vector.tensor_add(
                    el[:], el[:], ere_ps[:].rearrange("p (s h) -> p s h", h=4))
                nc.scalar.activation(el[:], el[:], mybir.ActivationFunctionType.Lrelu,
                                     alpha=NEG)
                nc.vector.tensor_mul(el[:], el[:],
                                     wc[:, :, None].to_broadcast([P, S, 4]))
                au = sb.tile([P, S, 4], BF16, tag="au")
                nc.scalar.activation(au[:], el[:], mybir.ActivationFunctionType.Exp)

                # msg = [ft * au | au]
                nc.vector.tensor_mul(
                    msg[:, :, 0:128].rearrange("p s (h d) -> p s h d", h=H),
                    g_t[:].rearrange("p s (h d) -> p s h d", h=H),
                    au[:, :, :, None].to_broadcast([P, S, 4, Dh]),
                )
                nc.vector.tensor_copy(msg[:, :, 128:132], au[:])

                if estage < 3:
                    xc0 = sb.tile([P, 128], F32, tag="xck")
                    nc.vector.tensor_copy(xc0[:], msg[:, 0, 0:128])
                    nc.sync.dma_start(out_local[ch0 * P:(ch0 + 1) * P, :], xc0[:])
                    s_off += S; lo_off += nlo; hi_off += nhi; ch0 += g
                    continue

                # --- scatter + epilogue per chunk ---
                for cg in range(g):
                    ch = ch0 + cg
                    psc = ps_e.tile([P, 132], F32, space="PSUM", tag="psc")
                    for t in range(T):
                        if t < T_LO:
                            s = cg * T_LO + t
                        else:
                            s = g * T_LO + cg * T_HI + (t - T_LO)
                        nc.tensor.matmul(
                            out=psc[:], lhsT=oh[:, s, :], rhs=msg[:, s, :],
                            start=(t == 0), stop=(t == T - 1),
                        )
                    s4 = sb.tile([P, 4], F32, tag="s4")
                    nc.vector.tensor_scalar_max(s4[:], psc[:, 128:132], 1e-30)
                    rinv = sb.tile([P, 4], F32, tag="rinv")
                    nc.vector.reciprocal(rinv[:], s4[:])
                    xc = sb.tile([P, 128], F32, tag="xck")
                    nc.vector.tensor_mul(
                        xc[:].rearrange("p (h d) -> p h d", h=H),
                        psc[:, 0:128].rearrange("p (h d) -> p h d", h=H),
                        rinv[:, :, None].to_broadcast([P, 4, Dh]),
                    )
                    nc.scalar.activation(xc[:], xc[:],
                                         mybir.ActivationFunctionType.Relu)
                    # transpose x chunk -> [feat, node] bf16
                    xt_ps = ps_ft.tile([P, P], F32, space="PSUM", tag="psb")
                    nc.tensor.transpose(xt_ps[:], xc[:], ident_t[:])
                    xTb = sb_ft.tile([P, P], BF16, tag="xTb")
                    nc.scalar.activation(xTb[:], xt_ps[:],
                                         mybir.ActivationFunctionType.Copy)
                    if not do_head:
                        kblk = blk_of_ch[ch]
                        coff = (ch - blk_ch0[kblk]) * P
                        nc.sync.dma_start(x1T_k[kblk][:, coff:coff + P],
                                          xTb[:])
                        if fuse_ft2 is not None:
                            we2, er2 = fuse_ft2
                            psf2 = ps_ft.tile([P, 4], F32, space="PSUM",
                                              tag="psf", name=f"psf2_{ch}")
                            nc.tensor.matmul(out=psf2[:], lhsT=xTb[:],
                                             rhs=we2[:, 128:132],
                                             start=True, stop=True)
                            nc.vector.tensor_copy(er2[:, ch, :], psf2[:])
                    else:
                        # fused head: out = x1 @ Wtop + x2 @ Wbot + bout
                        kblk = blk_of_ch[ch]
                        coff = (ch - blk_ch0[kblk]) * P
                        x1tc = sb_ft.tile([P, P], BF16, tag="x1tc")
                        nc.sync.dma_start(x1tc[:], x1T_k[kblk][:, coff:coff + P])
                        pso = ps_ft.tile([P, 132], F32, space="PSUM", tag="psf",
                                         name=f"pso_{layer}_{ch}")[:, 0:128]
                        nc.tensor.matmul(out=pso[:], lhsT=x1tc[:], rhs=wout_t[0][:],
                                         start=True, stop=False)
                        nc.tensor.matmul(out=pso[:], lhsT=xTb[:], rhs=wout_t[1][:],
                                         start=False, stop=False)
                        nc.tensor.matmul(out=pso[:], lhsT=ones_t[:], rhs=bout_t[:],
                                         start=False, stop=True)
                        oc = sb_ft.tile([P, 128], F32, tag="oc")
                        nc.scalar.activation(oc[:], pso[:],
                                             mybir.ActivationFunctionType.Copy)
                        nc.sync.dma_start(out_local[ch * P:(ch + 1) * P, :], oc[:])
                if not do_head and gi == blk_groups[grp_block[gi]][-1]:
                    k = grp_block[gi]
                    if os.environ.get("KNOAG"):
                        nc.gpsimd.dma_start(x1Tf_k[k][0:128, :],
                                            x1T_k[k][:, :])
                    else:
                        nc.gpsimd.collective_compute(
                            "AllGather", mybir.AluOpType.bypass,
                            replica_groups=rg,
                            ins=[x1T_k[k][:]], outs=[x1Tf_k[k][:]],
                        )
                s_off += S
                lo_off += nlo
                hi_off += nhi
                ch0 += g

        def ft2_full_phase(we2):
            """Replicated ft2: rebuild the full table from gathered x1T."""
            BCH = 3
            mblk = max(blk_nch)
            for k in range(nspl):
                nchk = blk_nch[k]
                for c in range(NCORE):
                    slab = sb_ft.tile([P, mblk * P], BF16, tag="slab0", bufs=2)
                    nc.sync.dma_start(slab[:, 0:nchk * P],
                                      x1Tf_k[k][c * P:(c + 1) * P, :])
                    for c0 in range(0, nchk, BCH):
                        nb = min(BCH, nchk - c0)
                        psb = ps_ft.tile([P, BCH, 132], F32, space="PSUM",
                                         tag="psb")
                        for ci in range(nb):
                            nc.tensor.matmul(
                                out=psb[:, ci, 0:128],
                                lhsT=slab[:, (c0 + ci) * P:(c0 + ci + 1) * P],
                                rhs=we2[:, 0:128], start=True, stop=True)
                        ftb = sb_ft.tile([P, BCH, 128], BF16, tag="ftb")
                        nc.scalar.activation(
                            ftb[:, 0:nb, :], psb[:, 0:nb, 0:128],
                            mybir.ActivationFunctionType.Copy)
                        r0 = c * nlp + (blk_ch0[k] + c0) * P
                        nc.sync.dma_start(
                            ftf2[r0:r0 + nb * P, :].rearrange(
                                "(c p) f -> p c f", c=nb),
                            ftb[:, 0:nb, :])

        bisect = os.environ.get("KBISECT", "")
        KREP = int(os.environ.get("KREP", "1"))
        for _rep in range(KREP):
            # ================= layer 1 (replicated ft, no collective) ======
            er1_phase(kin1, er_st[0])
            ft1_full_phase(kin1, w1e_t)
            if bisect == "ft":
                nc.gpsimd.dma_start(out_local[:, :], ftf1[0:nlp, :])
            if not bisect or bisect in ("l1", "l2"):
                edge_phase(1, ftf1, er_st[0], al_t[0], do_head=False,
                           fuse_ft2=(w2e_t, er_st[1]))

            # ================= layer 2 (head fused) =================
            if not bisect or bisect == "l2":
                ft2_full_phase(w2e_t)
                edge_phase(2, ftf2, er_st[1], al_t[1], do_head=(not bisect))
                if bisect == "l2":
                    nc.gpsimd.dma_start(out_local[:, :], ftf2[0:nlp, :])

    nc.compile()
    return nc


# ----------------------------------------------------------------------------
# public entry point
# ----------------------------------------------------------------------------
def _prepare(features, src, dst, w, W1, al1, ar1, W2, al2, ar2, Wout, bout):
    import ml_dtypes
    n_nodes = features.shape[0]
    kin1 = features.shape[1]
    nloc = math.ceil(n_nodes / NCORE)            # 6250
    nlp = math.ceil(nloc / P) * P                # 6272
    G = int(os.environ.get("KG", "4"))

    features = np.asarray(features, np.float32)
    src = np.asarray(src, np.int64)
    dst = np.asarray(dst, np.int64)
    w = np.asarray(w, np.float32)

    plan = _edge_plan(src, dst, w, n_nodes, nloc, nlp, G)
    T = plan["T_LO"] + plan["T_HI"]
    slot_of = plan["slot_of"]

    def ext(W, ar):
        wr = (np.asarray(W, np.float32).reshape(W.shape[0], H, Dh)
              * np.asarray(ar, np.float32)[None]).sum(-1)
        return np.concatenate([np.asarray(W, np.float32), wr], axis=1)

    w1e = ext(W1, ar1).astype(ml_dtypes.bfloat16)
    w2e = ext(W2, ar2).astype(ml_dtypes.bfloat16)
    al1f = np.asarray(al1, np.float32).reshape(-1)
    al2f = np.asarray(al2, np.float32).reshape(-1)
    al1rep = np.tile(al1f[None, :], (P, T)).astype(ml_dtypes.bfloat16)
    al2rep = np.tile(al2f[None, :], (P, T)).astype(ml_dtypes.bfloat16)
    iota_np = np.tile(np.arange(P, dtype=np.float32)[None, :], (P, 1)).astype(
        ml_dtypes.bfloat16)
    iotac_np = np.arange(P, dtype=np.float32)[:, None].astype(ml_dtypes.bfloat16)

    common = dict(
        w1e=w1e, w2e=w2e,
        wout=np.asarray(Wout, np.float32).astype(ml_dtypes.bfloat16),
        boutr=np.asarray(bout, np.float32).reshape(1, 128).astype(ml_dtypes.bfloat16),
        al1r=al1rep, al2r=al2rep, iota=iota_np, iotac=iotac_np,
        onesr=np.ones((1, 128), ml_dtypes.bfloat16),
    )
    # full slot-ordered feature table, transposed -- identical on every core
    x0p = np.zeros((NCORE * nlp, kin1), np.float32)
    x0p[slot_of[:n_nodes]] = features[:n_nodes]
    x0T_full = np.ascontiguousarray(x0p.T).astype(ml_dtypes.bfloat16)
    in_maps = []
    for c in range(NCORE):
        pc = plan["cores"][c]
        in_maps.append(dict(
            x0T=x0T_full,
            x0Tl=np.ascontiguousarray(x0T_full[:, c * nlp:(c + 1) * nlp]),
            srclo=pc["srclo_w"], srchi=pc["srchi_w"],
            dstrel=pc["dstrel"], wcol=pc["wcol"], dstrelT=pc["dstrelT"],
            **common,
        ))

    prog = _build_program(n_nodes, nloc, nlp, kin1, plan)
    return prog, in_maps, (n_nodes, nloc, nlp, slot_of)


def _run(features, src, dst, w, W1, al1, ar1, W2, al2, ar2, Wout, bout,
         trace=False):
    prog, in_maps, (n_nodes, nloc, nlp, slot_of) = _prepare(
        features, src, dst, w, W1, al1, ar1, W2, al2, ar2, Wout, bout)
    global _LAST_PROG
    _LAST_PROG = (prog, in_maps)
    res = run_bass_kernel_spmd(prog, in_maps, list(range(NCORE)), trace=trace)

    full = np.zeros((n_nodes, 128), np.float32)
    for c in range(NCORE):
        lo = c * nloc
        hi = min((c + 1) * nloc, n_nodes)
        full[lo:hi] = res.results[c]["out_local"][slot_of[lo:hi] - c * nlp]
    return full, res


def kernel(features, src, dst, w, W1, al1, ar1, W2, al2, ar2, Wout, bout):
    out, _ = _run(features, src, dst, w, W1, al1, ar1, W2, al2, ar2, Wout, bout)
    return out



# revision 15
# speedup vs baseline: 2.4014x; 2.4014x over previous
"""GAT (2-layer graph attention + output projection) on 8 Trainium2 NeuronCores.

v2. Sharding: destination nodes partitioned across cores (contiguous ranges of
6250), then *permuted within each core* by a balanced bin-packing so that every
128-node chunk has <=768 incoming lo-edges and <=384 hi-edges (lo/hi = source
row < / >= 32768 in the padded global table; int16 gather indices force the
split). This gives a uniform T_LO=6 / T_HI=3 tile structure (441 tiles/core,
~12% pad) vs the naive layout's (6,4) = 490 tiles.

Per layer: ft = x @ [W | wr] per local chunk (bf16, fed by transposed x tables
so no PE transposes); er (4 attention cols) stays in SBUF. ft tables are
AllGathered, each core dma_gathers ft[src] rows for its edges (two gathers,
lo/hi, striped over 2 SWDGE queues). Per-edge er[dst] is reconstructed on-chip:
dstrel values are partition-broadcast + compared against a partition-iota to
build transposed one-hots, and a [128x128]@[128x4] matmul per tile broadcasts
the chunk's er into edge slots -- this removes the v1 per-edge er dma_gather
(half of all descriptors). Segment softmax runs without max subtraction;
messages scatter into per-chunk PSUM via one-hot matmuls; the output head is
fused into the layer-2 epilogue using the transposed x1/x2 tiles directly.
"""
import math
import os
import numpy as np

import concourse.bass as bass
import concourse.tile as tile
from concourse import bacc, mybir
from concourse.bass_utils import run_bass_kernel_spmd
from concourse.masks import make_identity
from contextlib import ExitStack

F32 = mybir.dt.float32
BF16 = mybir.dt.bfloat16
I16 = mybir.dt.int16

NCORE = 8
H, Dh = 4, 32
NEG = 0.1
HALF = 32768
P = 128
PAD_DST = 1000.0


# ----------------------------------------------------------------------------
# host-side graph preprocessing
# ----------------------------------------------------------------------------
def _pack_bins(deg_lo, deg_hi, nbins, cap_lo, cap_hi):
    """Greedy balanced packing of nodes into nbins chunks of <=128 nodes with
    per-chunk lo/hi edge caps. Returns bin id per node, or None if infeasible."""
    n = len(deg_lo)
    order = np.argsort(-(deg_lo + deg_hi), kind="stable")
    rem_lo = np.full(nbins, cap_lo, np.int64)
    rem_hi = np.full(nbins, cap_hi, np.int64)
    cnt = np.zeros(nbins, np.int64)
    binid = np.full(n, -1, np.int64)
    for i in order:
        l, h = deg_lo[i], deg_hi[i]
        feas = (rem_lo >= l) & (rem_hi >= h) & (cnt < P)
        if not feas.any():
            return None
        score = np.where(feas, rem_lo + rem_hi, -1)
        b = int(np.argmax(score))
        binid[i] = b
        rem_lo[b] -= l
        rem_hi[b] -= h
        cnt[b] += 1
    return binid


def _edge_plan(src, dst, w, n_nodes, nloc, nlp, G):
    """Balanced-packed edge plan. Returns per-core gather/scatter arrays and
    the node permutation (slot_of: global node -> padded global row)."""
    nch = nlp // P
    npad = NCORE * nlp
    core_of_node = np.minimum(np.arange(n_nodes) // nloc, NCORE - 1)
    loc_of_node = np.arange(n_nodes) - core_of_node * nloc

    # core 5 straddles the HALF boundary: pin its first `pin` nodes (by local
    # id) to chunks [0, pin//P) so lo/hi classification is permutation-free.
    pin_core = HALF // nlp            # 5
    pin = HALF - pin_core * nlp       # 1408
    lo_src_bound = pin_core * nloc + pin   # global id threshold for lo edges

    edge_lo = src < lo_src_bound
    dst_core = dst // nloc

    # per-core packing -> slot_of (global node id -> padded global row)
    slot_of = np.zeros(n_nodes, np.int64)
    T_LO_req, T_HI_req = 6, 3
    for c in range(NCORE):
        lo_ = c * nloc
        hi_ = min((c + 1) * nloc, n_nodes)
        nn = hi_ - lo_
        deg_lo = np.bincount(dst[(dst_core == c) & edge_lo] - lo_, minlength=nn)
        deg_hi = np.bincount(dst[(dst_core == c) & ~edge_lo] - lo_, minlength=nn)
        for caps in ((6, 3), (7, 4), (8, 5)):
            cl, chh = caps[0] * P, caps[1] * P
            if c == pin_core:
                nb0 = pin // P
                b0 = _pack_bins(deg_lo[:pin], deg_hi[:pin], nb0, cl, chh)
                b1 = _pack_bins(deg_lo[pin:], deg_hi[pin:], nch - nb0, cl, chh)
                if b0 is None or b1 is None:
                    continue
                binid = np.concatenate([b0, b1 + nb0])
            else:
                binid = _pack_bins(deg_lo, deg_hi, nch, cl, chh)
                if binid is None:
                    continue
            T_LO_req = max(T_LO_req, caps[0])
            T_HI_req = max(T_HI_req, caps[1])
            break
        else:
            raise RuntimeError("packing failed")
        # slots: nodes of bin b occupy rows b*P.. in assignment order
        slot = np.zeros(nn, np.int64)
        for b in range(nch):
            members = np.where(binid == b)[0]
            slot[members] = b * P + np.arange(len(members))
        slot_of[lo_:hi_] = c * nlp + slot

    T_LO, T_HI = T_LO_req, T_HI_req
    T = T_LO + T_HI
    srcpad = slot_of[src]
    dstpad = slot_of[dst]

    groups = [G] * (nch // G) + ([nch % G] if nch % G else [])
    S_tot = sum(g * T for g in groups)
    LO_tot = sum(g * T_LO for g in groups) * P
    HI_tot = sum(g * T_HI for g in groups) * P

    def wrap(flat):
        a = flat.reshape(-1, 16).T.copy()
        return np.tile(a, (8, 1)).astype(np.int16)

    def group_offsets():
        offs = []
        s_off = lo_off = hi_off = 0
        for g in groups:
            offs.append((s_off, lo_off, hi_off, g))
            s_off += g * T
            lo_off += g * T_LO * P
            hi_off += g * T_HI * P
        return offs

    out = []
    for c in range(NCORE):
        m = dst_core == c
        e_ids = np.where(m)[0]
        e_chunk = (dstpad[e_ids] - c * nlp) // P
        e_rel = (dstpad[e_ids] - c * nlp) % P
        e_lo = edge_lo[e_ids]

        srclo = np.zeros(LO_tot, np.int64)
        srchi = np.zeros(HI_tot, np.int64)
        dstrel = np.full((P, S_tot), PAD_DST, np.float32)
        wcol = np.zeros((P, S_tot), np.float32)

        # bucket edges per (chunk, lo/hi)
        order = np.argsort(e_chunk * 2 + (~e_lo).astype(np.int64), kind="stable")
        eo = e_ids[order]
        co = e_chunk[order]
        ro = e_rel[order]
        loo = e_lo[order]

        s_off = lo_off = hi_off = 0
        ch0 = 0
        ck_lo_idx = [np.where((co == ch) & loo)[0] for ch in range(nch)]
        ck_hi_idx = [np.where((co == ch) & ~loo)[0] for ch in range(nch)]

        for g in groups:
            for cg in range(g):
                ch = ch0 + cg
                for is_lo, idxs, Treg, roff, arr, rbase in (
                    (True, ck_lo_idx[ch], T_LO, lo_off, srclo, 0),
                    (False, ck_hi_idx[ch], T_HI, hi_off, srchi, HALF),
                ):
                    for j, ei in enumerate(idxs):
                        t, p = j // P, j % P
                        if is_lo:
                            i_flat = roff + (cg * Treg + t) * P + p
                            arr[i_flat] = srcpad[eo[ei]] - rbase
                            s = s_off + cg * T_LO + t
                        else:
                            i_flat = roff + (cg * Treg + t) * P + p
                            arr[i_flat] = srcpad[eo[ei]] - rbase
                            s = s_off + g * T_LO + cg * T_HI + t
                        dstrel[p, s] = float(ro[ei])
                        wcol[p, s] = w[eo[ei]]
            s_off += g * T
            lo_off += g * T_LO * P
            hi_off += g * T_HI * P
            ch0 += g

        import ml_dtypes
        # host-precomputed one-hots (bf16): oh[q, s, i] = [dstrel[q,s] == i],
        # ohT[p, s, q] = [dstrel[q,s] == p]; PAD_DST rows are all-zero
        idx = np.arange(P, dtype=np.float32)
        ohb = (dstrel[:, :, None] == idx[None, None, :])
        ohTb = np.ascontiguousarray(ohb.transpose(2, 1, 0))
        # interleave per group: [oh_g | ohT_g] so one DMA fetches both
        ohcat = []
        srcj = []
        for (s0, lo0, hi0, g) in group_offsets():
            S = g * T
            ohcat.append(ohb[:, s0:s0 + S, :].reshape(P, -1))
            ohcat.append(ohTb[:, s0:s0 + S, :].reshape(P, -1))
            srcj.append(srclo[lo0:lo0 + g * T_LO * P])
            srcj.append(srchi[hi0:hi0 + g * T_HI * P])
        out.append(dict(
            srcj_w=wrap(np.concatenate(srcj)),
            wcol=wcol.astype(ml_dtypes.bfloat16),
            ohcat=np.concatenate(ohcat, axis=1).astype(ml_dtypes.bfloat16),
        ))
    return dict(T_LO=T_LO, T_HI=T_HI, groups=groups, cores=out, slot_of=slot_of,
                S_tot=S_tot, LO_tot=LO_tot, HI_tot=HI_tot)


# ----------------------------------------------------------------------------
# device program
# ----------------------------------------------------------------------------
def _build_program(n_nodes, nloc, nlp, kin1, plan):
    T_LO, T_HI, groups = plan["T_LO"], plan["T_HI"], plan["groups"]
    T = T_LO + T_HI
    nch = nlp // P
    npad = NCORE * nlp

    nswq = int(os.environ.get("KNSWQ", "2"))
    nc = bacc.Bacc("TRN2", target_bir_lowering=False, debug=False,
                   num_devices=NCORE, num_swdge_queues=nswq)

    # ---- I/O ----
    # layer-1 ft is computed REPLICATED (every core computes the full table
    # from the full feature matrix) -- 43us of PE work replaces a ~730us
    # AllGather. x0T is the full slot-ordered feature table (same on every
    # core); x0Tl is this core's slice (only used for the tiny er1 pass,
    # which must read core-local columns in an SPMD-uniform way).
    x0T = nc.dram_tensor("x0T", [kin1, NCORE * nlp], BF16,
                         kind="ExternalInput")
    x0Tl = nc.dram_tensor("x0Tl", [kin1, nlp], BF16, kind="ExternalInput")
    w1e = nc.dram_tensor("w1e", [kin1, 132], BF16, kind="ExternalInput")
    w2e = nc.dram_tensor("w2e", [128, 132], BF16, kind="ExternalInput")
    wout = nc.dram_tensor("wout", [256, 128], BF16, kind="ExternalInput")
    boutr = nc.dram_tensor("boutr", [1, 128], BF16, kind="ExternalInput")
    al1r = nc.dram_tensor("al1r", [P, T * P], BF16, kind="ExternalInput")
    al2r = nc.dram_tensor("al2r", [P, T * P], BF16, kind="ExternalInput")
    iota = nc.dram_tensor("iota", [P, P], BF16, kind="ExternalInput")
    iotac = nc.dram_tensor("iotac", [P, 1], BF16, kind="ExternalInput")
    ohd = nc.dram_tensor("ohd", [P, 2 * plan["S_tot"] * P], BF16,
                         kind="ExternalInput")
    onesr = nc.dram_tensor("onesr", [1, 128], BF16, kind="ExternalInput")
    srcj = nc.dram_tensor(
        "srcj", [P, (plan["LO_tot"] + plan["HI_tot"]) // 16], I16,
        kind="ExternalInput")
    wcol = nc.dram_tensor("wcol", [P, plan["S_tot"]], BF16, kind="ExternalInput")
    out_local = nc.dram_tensor("out_local", [nlp, 128], F32, kind="ExternalOutput")

    # ---- internal DRAM ----
    ftl2 = nc.dram_tensor("ft2_local", [nlp, 128], BF16)
    ftf1 = nc.dram_tensor("ft1_full", [npad, 128], BF16)
    ftf2 = nc.dram_tensor("ft2_full", [npad, 128], BF16, addr_space="Shared")
    x1T_d = nc.dram_tensor("x1T", [128, nlp], BF16)

    rg = [list(range(NCORE))]

    with tile.TileContext(nc) as tc, ExitStack() as ctx:
        consts = ctx.enter_context(tc.tile_pool(name="consts", bufs=1))
        sb = ctx.enter_context(tc.tile_pool(
            name="sb", bufs=int(os.environ.get("KSBUFS", "3"))))
        sb_ft = ctx.enter_context(tc.tile_pool(name="sb_ft", bufs=3))
        ps_ft = ctx.enter_context(tc.tile_pool(name="ps_ft", bufs=2, space="PSUM"))
        ps_e = ctx.enter_context(tc.tile_pool(name="ps_e", bufs=2, space="PSUM"))

        # ---- constants in SBUF ----
        iota_t = consts.tile([P, P], BF16)
        nc.sync.dma_start(iota_t[:], iota[:, :])
        iotac_t = consts.tile([P, 1], BF16)
        nc.sync.dma_start(iotac_t[:], iotac[:, :])
        ones_t = consts.tile([1, 128], BF16)
        nc.sync.dma_start(ones_t[:], onesr[:, :])
        bout_t = consts.tile([1, 128], BF16)
        nc.sync.dma_start(bout_t[:], boutr[:, :])
        w1e_t = [consts.tile([P, 132], BF16, name=f"w1e{k}", tag=f"w1e{k}")
                 for k in range(kin1 // P)]
        for k in range(kin1 // P):
            nc.sync.dma_start(w1e_t[k][:], w1e[k * P:(k + 1) * P, :])
        w2e_t = consts.tile([P, 132], BF16)
        nc.sync.dma_start(w2e_t[:], w2e[:, :])
        wout_t = [consts.tile([P, 128], BF16, name=f"wout{k}", tag=f"wout{k}")
                  for k in range(2)]
        for k in range(2):
            nc.sync.dma_start(wout_t[k][:], wout[k * P:(k + 1) * P, :])
        al_t = []
        for l, src_al in ((0, al1r), (1, al2r)):
            a = consts.tile([P, T * P], BF16, name=f"al{l}", tag=f"al{l}")
            nc.sync.dma_start(a[:], src_al[:, :])
            al_t.append(a)
        er_st = [consts.tile([P, nch, 4], BF16, name=f"er{l}", tag=f"er{l}")
                 for l in (1, 2)]
        ident_t = consts.tile([P, P], F32)
        make_identity(nc, ident_t[:])
        identb_t = consts.tile([P, P], BF16)
        make_identity(nc, identb_t[:])

        def er1_phase(kin, er_tile):
            """er1 for this core's local chunks: x0Tl @ w1e[:, 128:132]."""
            EB = 4
            for cb in range(0, nch, EB):
                nbc = min(EB, nch - cb)
                xts = []
                for k in range(kin // P):
                    xt = sb_ft.tile([P, EB * P], BF16, tag="erxt", bufs=4)
                    nc.sync.dma_start(
                        xt[:, 0:nbc * P],
                        x0Tl[k * P:(k + 1) * P, cb * P:(cb + nbc) * P])
                    xts.append(xt)
                for ci in range(nbc):
                    pse = ps_ft.tile([P, 4], F32, space="PSUM", tag="psf")
                    for k in range(kin // P):
                        nc.tensor.matmul(
                            out=pse[:], lhsT=xts[k][:, ci * P:(ci + 1) * P],
                            rhs=w1e_t[k][:, 128:132],
                            start=(k == 0), stop=(k == kin // P - 1))
                    nc.vector.tensor_copy(er_tile[:, cb + ci, :], pse[:])

        def ft1_full_phase(kin, we_tiles):
            """Full replicated ft1 table: every core computes all npad rows."""
            nchg = npad // P            # 392 global chunks
            BLKC = 14                   # chunks per slab load (28 blocks)
            BCH = 3                     # chunks per PSUM batch (<=512 f32/bank)
            for b0 in range(0, nchg, BLKC):
                nbk = min(BLKC, nchg - b0)
                slabs = []
                for k in range(kin // P):
                    s = sb_ft.tile([P, BLKC * P], BF16, tag=f"slab{k}", bufs=2)
                    nc.sync.dma_start(
                        s[:, 0:nbk * P],
                        x0T[k * P:(k + 1) * P, b0 * P:(b0 + nbk) * P])
                    slabs.append(s)
                for c0 in range(0, nbk, BCH):
                    nb = min(BCH, nbk - c0)
                    psb = ps_ft.tile([P, BCH, 132], F32, space="PSUM",
                                     tag="psb")
                    for ci in range(nb):
                        for k in range(kin // P):
                            nc.tensor.matmul(
                                out=psb[:, ci, :],
                                lhsT=slabs[k][:, (c0 + ci) * P:(c0 + ci + 1) * P],
                                rhs=we_tiles[k][:],
                                start=(k == 0), stop=(k == kin // P - 1))
                    ftb = sb_ft.tile([P, BCH, 128], BF16, tag="ftb")
                    nc.scalar.activation(ftb[:, 0:nb, :], psb[:, 0:nb, 0:128],
                                         mybir.ActivationFunctionType.Copy)
                    r0 = (b0 + c0) * P
                    nc.sync.dma_start(
                        ftf1[r0:r0 + nb * P, :].rearrange(
                            "(c p) f -> p c f", c=nb),
                        ftb[:, 0:nb, :])

        def edge_phase(layer, ftf_d, er_tile, al_rep, do_head=False,
                       fuse_ft2=None):
            estage = int(os.environ.get("KEDGE", "9"))
            s_off = lo_off = hi_off = 0
            ch0 = 0
            for gi, g in enumerate(groups):
                S = g * T
                nlo, nhi = g * T_LO * P, g * T_HI * P
                j_off = (lo_off + hi_off) // 16
                # --- edge metadata loads (merged [srclo|srchi] block) ---
                sj = sb.tile([P, (nlo + nhi) // 16], I16, tag="ilo")
                nc.sync.dma_start(
                    sj[:], srcj[:, j_off:j_off + (nlo + nhi) // 16])
                ilo = sj[:, 0:nlo // 16]
                ihi = sj[:, nlo // 16:(nlo + nhi) // 16]
                wc = sb.tile([P, S], BF16, tag="wc")
                nc.sync.dma_start(wc[:], wcol[:, s_off:s_off + S])


                # --- ft[src] gathers (2 queues) ---
                g_t = sb.tile([P, S, 128], BF16, tag="g_t",
                              bufs=int(os.environ.get("KGBUFS", "3")))
                nq = 1 if os.environ.get("KNOQ2") else nswq
                nc.gpsimd.dma_gather(
                    out_ap=g_t[:, 0:g * T_LO, :], in_ap=ftf_d[:, :],
                    idxs_ap=ilo, num_idxs=nlo, num_idxs_reg=nlo, elem_size=128,
                    single_packet=False, queue_num=(2 * gi) % nq,
                )
                nc.gpsimd.dma_gather(
                    out_ap=g_t[:, g * T_LO:S, :], in_ap=ftf_d[HALF:npad, :],
                    idxs_ap=ihi, num_idxs=nhi, num_idxs_reg=nhi, elem_size=128,
                    single_packet=False, queue_num=(2 * gi + 1) % nq,
                )

                # --- one-hots: host-precomputed, one merged DMA ---
                if True:
                    obt = sb.tile([P, 2 * S, P], BF16, tag="oh")
                    nc.sync.dma_start(
                        obt[:].rearrange("p s q -> p (s q)"),
                        ohd[:, 2 * s_off * P:2 * (s_off + S) * P])
                    oh = obt[:, 0:S, :]
                    ohT = obt[:, S:2 * S, :]

                if estage < 2:
                    xc0 = sb.tile([P, 128], F32, tag="xck")
                    nc.vector.tensor_copy(xc0[:], g_t[:, 0, :])
                    nc.sync.dma_start(out_local[ch0 * P:(ch0 + 1) * P, :], xc0[:])
                    s_off += S; lo_off += nlo; hi_off += nhi; ch0 += g
                    continue

                # --- er broadcast into edge slots: ere[p, s*4+h] ---
                ere_ps = ps_e.tile([P, S * 4], F32, space="PSUM", tag="ere")
                for cg in range(g):
                    for t in range(T):
                        if t < T_LO:
                            s = cg * T_LO + t
                        else:
                            s = g * T_LO + cg * T_HI + (t - T_LO)
                        nc.tensor.matmul(
                            out=ere_ps[:, s * 4:(s + 1) * 4],
                            lhsT=ohT[:, s, :], rhs=er_tile[:, ch0 + cg, :],
                            start=True, stop=True,
                        )

                # --- el = <ft, al> per head (elp scratch aliases msg cols) ---
                msg = sb.tile([P, S, 132], BF16, tag="msg",
                              bufs=int(os.environ.get("KGBUFS", "3")))
                elp = msg[:, :, 0:128]
                for cg in range(g):
                    nc.vector.tensor_mul(
                        elp[:, cg * T:(cg + 1) * T, :],
                        g_t[:, cg * T:(cg + 1) * T, :],
                        al_rep[:].rearrange("p (t j) -> p t j", j=P),
                    )
                el = sb.tile([P, S, 4], F32, tag="el")
                nc.vector.reduce_sum(
                    el[:],
                    elp[:].rearrange("p s (h d) -> p s h d", h=H),
                    axis=mybir.AxisListType.X,
                )
                # logit = lrelu(el + er) * w ; au = exp(logit)
                nc.vector.tensor_add(
                    el[:], el[:], ere_ps[:].rearrange("p (s h) -> p s h", h=4))
                nc.scalar.activation(el[:], el[:], mybir.ActivationFunctionType.Lrelu,
                                     alpha=NEG)
                nc.vector.tensor_mul(el[:], el[:],
                                     wc[:, :, None].to_broadcast([P, S, 4]))
                au = sb.tile([P, S, 4], BF16, tag="au")
                nc.scalar.activation(au[:], el[:], mybir.ActivationFunctionType.Exp)

                # msg = [ft * au | au]
                nc.vector.tensor_mul(
                    msg[:, :, 0:128].rearrange("p s (h d) -> p s h d", h=H),
                    g_t[:].rearrange("p s (h d) -> p s h d", h=H),
                    au[:, :, :, None].to_broadcast([P, S, 4, Dh]),
                )
                nc.vector.tensor_copy(msg[:, :, 128:132], au[:])

                if estage < 3:
                    xc0 = sb.tile([P, 128], F32, tag="xck")
                    nc.vector.tensor_copy(xc0[:], msg[:, 0, 0:128])
                    nc.sync.dma_start(out_local[ch0 * P:(ch0 + 1) * P, :], xc0[:])
                    s_off += S; lo_off += nlo; hi_off += nhi; ch0 += g
                    continue

                # --- scatter + epilogue per chunk ---
                for cg in range(g):
                    ch = ch0 + cg
                    psc = ps_e.tile([P, 132], F32, space="PSUM", tag="psc")
                    for t in range(T):
                        if t < T_LO:
                            s = cg * T_LO + t
                        else:
                            s = g * T_LO + cg * T_HI + (t - T_LO)
                        nc.tensor.matmul(
                            out=psc[:], lhsT=oh[:, s, :], rhs=msg[:, s, :],
                            start=(t == 0), stop=(t == T - 1),
                        )
                    s4 = sb.tile([P, 4], F32, tag="s4")
                    nc.vector.tensor_scalar_max(s4[:], psc[:, 128:132], 1e-30)
                    rinv = sb.tile([P, 4], F32, tag="rinv")
                    nc.vector.reciprocal(rinv[:], s4[:])
                    xc = sb.tile([P, 128], F32, tag="xck")
                    nc.vector.tensor_mul(
                        xc[:].rearrange("p (h d) -> p h d", h=H),
                        psc[:, 0:128].rearrange("p (h d) -> p h d", h=H),
                        rinv[:, :, None].to_broadcast([P, 4, Dh]),
                    )
                    nc.scalar.activation(xc[:], xc[:],
                                         mybir.ActivationFunctionType.Relu)
                    # transpose x chunk -> [feat, node] bf16
                    xt_ps = ps_ft.tile([P, P], F32, space="PSUM", tag="psb")
                    nc.tensor.transpose(xt_ps[:], xc[:], ident_t[:])
                    xTb = sb_ft.tile([P, P], BF16, tag="xTb")
                    nc.scalar.activation(xTb[:], xt_ps[:],
                                         mybir.ActivationFunctionType.Copy)
                    if not do_head:
                        nc.sync.dma_start(x1T_d[:, ch * P:(ch + 1) * P], xTb[:])
                        if fuse_ft2 is not None:
                            we2, ftl2, er2 = fuse_ft2
                            psf2 = ps_ft.tile([P, 132], F32, space="PSUM",
                                              tag="psf", name=f"psf2_{ch}")
                            nc.tensor.matmul(out=psf2[:], lhsT=xTb[:],
                                             rhs=we2[:], start=True, stop=True)
                            ftb2 = sb_ft.tile([P, 128], BF16, tag="ftb")
                            nc.scalar.activation(
                                ftb2[:], psf2[:, 0:128],
                                mybir.ActivationFunctionType.Copy)
                            nc.sync.dma_start(ftl2[ch * P:(ch + 1) * P, :],
                                              ftb2[:])
                            nc.vector.tensor_copy(er2[:, ch, :],
                                                  psf2[:, 128:132])
                    else:
                        # fused head: out = x1 @ Wtop + x2 @ Wbot + bout
                        x1tc = sb_ft.tile([P, P], BF16, tag="x1tc")
                        nc.sync.dma_start(x1tc[:], x1T_d[:, ch * P:(ch + 1) * P])
                        pso = ps_ft.tile([P, 132], F32, space="PSUM", tag="psf",
                                         name=f"pso_{layer}_{ch}")[:, 0:128]
                        nc.tensor.matmul(out=pso[:], lhsT=x1tc[:], rhs=wout_t[0][:],
                                         start=True, stop=False)
                        nc.tensor.matmul(out=pso[:], lhsT=xTb[:], rhs=wout_t[1][:],
                                         start=False, stop=False)
                        nc.tensor.matmul(out=pso[:], lhsT=ones_t[:], rhs=bout_t[:],
                                         start=False, stop=True)
                        oc = sb_ft.tile([P, 128], F32, tag="oc")
                        nc.scalar.activation(oc[:], pso[:],
                                             mybir.ActivationFunctionType.Copy)
                        nc.sync.dma_start(out_local[ch * P:(ch + 1) * P, :], oc[:])
                s_off += S
                lo_off += nlo
                hi_off += nhi
                ch0 += g

        bisect = os.environ.get("KBISECT", "")
        KREP = int(os.environ.get("KREP", "1"))
        for _rep in range(KREP):
            # ================= layer 1 (replicated ft, no collective) ======
            er1_phase(kin1, er_st[0])
            ft1_full_phase(kin1, w1e_t)
            if bisect == "ft":
                nc.gpsimd.dma_start(out_local[:, :], ftf1[0:nlp, :])
            if not bisect or bisect in ("l1", "l2"):
                edge_phase(1, ftf1, er_st[0], al_t[0], do_head=False,
                           fuse_ft2=(w2e_t, ftl2, er_st[1]))

            # ================= layer 2 (head fused) =================
            if not bisect or bisect == "l2":
                if os.environ.get("KNOAG"):
                    nc.gpsimd.dma_start(ftf2[0:nlp, :], ftl2[:, :])
                else:
                    nc.gpsimd.collective_compute(
                        "AllGather", mybir.AluOpType.bypass, replica_groups=rg,
                        ins=[ftl2[:]], outs=[ftf2[:]],
                    )
                edge_phase(2, ftf2, er_st[1], al_t[1], do_head=(not bisect))
                if bisect == "l2":
                    nc.gpsimd.dma_start(out_local[:, :], ftl2[:, :])

    nc.compile()
    return nc


# ----------------------------------------------------------------------------
# public entry point
# ----------------------------------------------------------------------------
def _prepare(features, src, dst, w, W1, al1, ar1, W2, al2, ar2, Wout, bout):
    import ml_dtypes
    n_nodes = features.shape[0]
    kin1 = features.shape[1]
    nloc = math.ceil(n_nodes / NCORE)            # 6250
    nlp = math.ceil(nloc / P) * P                # 6272
    G = int(os.environ.get("KG", "4"))

    features = np.asarray(features, np.float32)
    src = np.asarray(src, np.int64)
    dst = np.asarray(dst, np.int64)
    w = np.asarray(w, np.float32)

    plan = _edge_plan(src, dst, w, n_nodes, nloc, nlp, G)
    T = plan["T_LO"] + plan["T_HI"]
    slot_of = plan["slot_of"]

    def ext(W, ar):
        wr = (np.asarray(W, np.float32).reshape(W.shape[0], H, Dh)
              * np.asarray(ar, np.float32)[None]).sum(-1)
        return np.concatenate([np.asarray(W, np.float32), wr], axis=1)

    w1e = ext(W1, ar1).astype(ml_dtypes.bfloat16)
    w2e = ext(W2, ar2).astype(ml_dtypes.bfloat16)
    al1f = np.asarray(al1, np.float32).reshape(-1)
    al2f = np.asarray(al2, np.float32).reshape(-1)
    al1rep = np.tile(al1f[None, :], (P, T)).astype(ml_dtypes.bfloat16)
    al2rep = np.tile(al2f[None, :], (P, T)).astype(ml_dtypes.bfloat16)
    iota_np = np.tile(np.arange(P, dtype=np.float32)[None, :], (P, 1)).astype(
        ml_dtypes.bfloat16)
    iotac_np = np.arange(P, dtype=np.float32)[:, None].astype(ml_dtypes.bfloat16)

    common = dict(
        w1e=w1e, w2e=w2e,
        wout=np.asarray(Wout, np.float32).astype(ml_dtypes.bfloat16),
        boutr=np.asarray(bout, np.float32).reshape(1, 128).astype(ml_dtypes.bfloat16),
        al1r=al1rep, al2r=al2rep, iota=iota_np, iotac=iotac_np,
        onesr=np.ones((1, 128), ml_dtypes.bfloat16),
    )
    # full slot-ordered feature table, transposed -- identical on every core
    x0p = np.zeros((NCORE * nlp, kin1), np.float32)
    x0p[slot_of[:n_nodes]] = features[:n_nodes]
    x0T_full = np.ascontiguousarray(x0p.T).astype(ml_dtypes.bfloat16)
    in_maps = []
    for c in range(NCORE):
        pc = plan["cores"][c]
        in_maps.append(dict(
            x0T=x0T_full,
            x0Tl=np.ascontiguousarray(x0T_full[:, c * nlp:(c + 1) * nlp]),
            srclo=pc["srclo_w"], srchi=pc["srchi_w"],
            dstrel=pc["dstrel"], wcol=pc["wcol"],
            ohd=pc["ohb"], ohTd=pc["ohTb"],
            **common,
        ))

    prog = _build_program(n_nodes, nloc, nlp, kin1, plan)
    return prog, in_maps, (n_nodes, nloc, nlp, slot_of)


def _run(features, src, dst, w, W1, al1, ar1, W2, al2, ar2, Wout, bout,
         trace=False):
    prog, in_maps, (n_nodes, nloc, nlp, slot_of) = _prepare(
        features, src, dst, w, W1, al1, ar1, W2, al2, ar2, Wout, bout)
    global _LAST_PROG
    _LAST_PROG = (prog, in_maps)
    res = run_bass_kernel_spmd(prog, in_maps, list(range(NCORE)), trace=trace)

    full = np.zeros((n_nodes, 128), np.float32)
    for c in range(NCORE):
        lo = c * nloc
        hi = min((c + 1) * nloc, n_nodes)
        full[lo:hi] = res.results[c]["out_local"][slot_of[lo:hi] - c * nlp]
    return full, res


def kernel(features, src, dst, w, W1, al1, ar1, W2, al2, ar2, Wout, bout):
    out, _ = _run(features, src, dst, w, W1, al1, ar1, W2, al2, ar2, Wout, bout)
    return out

